# revision 1
# baseline (speedup 1.0000x reference)
"""MoE feed-forward (top-2 of 8 experts) Trainium2 Bass kernel.

Problem: nn_MixtureOfExpertsFeedForward_6734508720763
  x[4,1024,1024] tokens, router Wr[1024,8], experts W_in[8,1024,4096],
  W_out[8,4096,1024], top_k=2.

  ref:  logits = x@Wr + br ; probs = softmax(logits)
        top2 -> dispatch (0/1), combine (prob or 0)
        h = sum_e dispatch[n,e] * relu(x @ W_in[e] + b_in[e])
        y = sum_e combine[n,e]  * (h @ W_out[e] + b_out[e])

Sharding: pure data parallel over the 4096 tokens -> 512 tokens/core on
8 cores, weights replicated, no collectives.

V1 strategy (dense over experts):
  - router matmul in true fp32 (top-2 pick must match the reference)
  - expert matmuls in float32r (FP22 single-pass, full PE rate at N=512)
    or fp16 (halves weight DMA traffic; host pre-casts weights)
  - per-expert masking folded into the ScalarE Relu via per-partition
    `scale` = dispatch mask (mask*relu(z) == relu(mask*z) for mask in {0,1})
  - h kept token-major, PE-transposed to hT for the second matmul
"""

import os
import sys

import numpy as np

sys.path.insert(0, "/opt/trn_rl_repo")

import concourse.bacc as bacc
import concourse.bass as bass
import concourse.mybir as mybir
import concourse.tile as tile
from concourse.bass_utils import run_bass_kernel_spmd

F32 = mybir.dt.float32
F32R = mybir.dt.float32r
F16 = mybir.dt.float16

P = 128          # partitions
NCORES = 8
N_TOK = 4096     # total tokens (4*1024)
T = N_TOK // NCORES   # tokens per core = 512
G = T // P       # token groups per core = 4
D = 1024
KD = D // P      # 8 contraction chunks for D
F = 4096
FC = F // 512    # 8 f-chunks of 512
FT = F // P      # 32 f-tiles of 128
E = 8
AX = mybir.AxisListType
AF = mybir.ActivationFunctionType
OP = mybir.AluOpType


def build_nc(cfg):
    """Build the single-core SPMD bass program.

    cfg keys: wdt ('f32r'|'f16') - dtype of expert weights + hT in matmuls;
              has_br/has_bin/has_bout - include bias adds.

    float32r note: the BIR verifier requires every buffer consumed by an
    FP32r matmul to be produced as float32r (DMA of a float32r-declared
    DRAM tensor, or an engine op with float32r output which rounds to
    FP22). numpy side stays float32 (same bytes; PE truncates on read).
    """
    wdt = F32R if cfg["wdt"] == "f32r" else F16
    w_store = F32R if cfg["wdt"] == "f32r" else F16
    has_br = cfg["has_br"]
    has_bin = cfg["has_bin"]
    has_bout = cfg["has_bout"]

    # Bacc (not plain Bass): its compile() runs the TRN2 legalization that
    # splits >1-sync-wait instructions (4-byte matmul LDW allows one wait).
    nc = bacc.Bacc(None)
    x_h = nc.declare_dram_parameter("x", [T, D], F32, isOutput=False)
    wr_h = nc.declare_dram_parameter("wr", [D, E], F32, isOutput=False)
    win_h = nc.declare_dram_parameter("w_in", [E, D, F], w_store, isOutput=False)
    wout_h = nc.declare_dram_parameter("w_out", [E, F, D], w_store, isOutput=False)
    br_h = nc.declare_dram_parameter("br", [1, E], F32, isOutput=False) if has_br else None
    bin_h = nc.declare_dram_parameter("b_in", [E, F], F32, isOutput=False) if has_bin else None
    bout_h = nc.declare_dram_parameter("b_out", [E, D], F32, isOutput=False) if has_bout else None
    y_h = nc.declare_dram_parameter("y", [T, D], F32, isOutput=True)

    with tile.TileContext(nc) as tc:
        with (
            tc.tile_pool(name="persist", bufs=1) as pp,
            tc.tile_pool(name="ps", bufs=6, space="PSUM") as psp,
        ):
            # ---- constants / persistent tiles ----
            ident = pp.tile([P, P], F32, tag="ident")
            from concourse.masks import make_identity
            make_identity(nc, ident[:])

            xT = pp.tile([P, KD, T], F32, tag="xT")          # x transposed, f32
            hT = pp.tile([P, FT, T], w_store, tag="hT")      # h transposed
            # mm1 lhsT in the matmul dtype (router keeps full-f32 xT)
            xTr = pp.tile([P, KD, T], w_store, tag="xTr", name="xTr")
            wr_sb = pp.tile([P, KD, E], F32, tag="wr")
            disp = pp.tile([P, G * E], F32, tag="disp")      # dispatch mask
            comb = pp.tile([P, G * E], F32, tag="comb")      # combine probs
            yac = [
                pp.tile([P, D], F32, tag=f"y{g}", name=f"yac{g}")
                for g in range(G)
            ]
            ones1 = pp.tile([1, P], F32, tag="ones1")
            if has_bin or has_bout:
                nc.vector.memset(ones1[:], 1.0)
            br_sb = None
            if has_br:
                br_sb = pp.tile([1, E], F32, tag="br")
                nc.sync.dma_start(br_sb[:], br_h[:])

            nc.sync.dma_start(
                wr_sb[:], wr_h[:, :].rearrange("(kd p) e -> p kd e", p=P)
            )

            # ---- load x, build xT via PE transpose ----
            with tc.tile_pool(name="xload", bufs=2) as xlp:
                for g in range(G):
                    xg = xlp.tile([P, D], F32, tag="xg")
                    nc.sync.dma_start(xg[:], x_h[g * P : (g + 1) * P, :])
                    for kd in range(KD):
                        pst = psp.tile([P, P], F32, tag="ps")
                        nc.tensor.transpose(
                            pst[:], xg[:, kd * P : (kd + 1) * P], ident[:]
                        )
                        nc.vector.tensor_copy(
                            xT[:, kd, g * P : (g + 1) * P], pst[:]
                        )
                        nc.vector.tensor_copy(
                            xTr[:, kd, g * P : (g + 1) * P], pst[:]
                        )

            # ---- router (true fp32 matmul; top-2 must match reference) ----
            with tc.tile_pool(name="rt", bufs=2) as rtp:
                for g in range(G):
                    psr = psp.tile([P, E], F32, tag="ps")
                    for kd in range(KD):
                        nc.tensor.matmul(
                            psr[:],
                            lhsT=xT[:, kd, g * P : (g + 1) * P],
                            rhs=wr_sb[:, kd, :],
                            start=(kd == 0),
                            stop=(kd == KD - 1 and not has_br),
                        )
                    if has_br:
                        nc.tensor.matmul(
                            psr[:], lhsT=ones1[:, :], rhs=br_sb[:, :],
                            start=False, stop=True,
                        )
                    lg = rtp.tile([P, E], F32, tag="lg")
                    nc.vector.tensor_copy(lg[:], psr[:])
                    mx1 = rtp.tile([P, 1], F32, tag="mx1")
                    nmx = rtp.tile([P, 1], F32, tag="nmx")
                    nc.vector.reduce_max(out=mx1[:], in_=lg[:], axis=AX.X)
                    nc.vector.reduce_max(out=nmx[:], in_=lg[:], axis=AX.X, negate=True)
                    is1 = rtp.tile([P, E], F32, tag="is1")
                    nc.vector.tensor_scalar(
                        out=is1[:], in0=lg[:], scalar1=mx1[:, :1], scalar2=None,
                        op0=OP.is_equal,
                    )
                    lgm = rtp.tile([P, E], F32, tag="lgm")
                    nc.vector.tensor_scalar_mul(is1[:], is1[:], 1e30)
                    nc.vector.tensor_sub(lgm[:], lg[:], is1[:])
                    mx2 = rtp.tile([P, 1], F32, tag="mx2")
                    nc.vector.reduce_max(out=mx2[:], in_=lgm[:], axis=AX.X)
                    dcol = disp[:, g * E : (g + 1) * E]
                    nc.vector.tensor_scalar(
                        out=dcol, in0=lg[:], scalar1=mx2[:, :1], scalar2=None,
                        op0=OP.is_ge,
                    )
                    # softmax over all 8 then mask by dispatch
                    ex = rtp.tile([P, E], F32, tag="ex")
                    nc.scalar.activation(ex[:], lg[:], AF.Exp, bias=nmx[:, :1])
                    sm = rtp.tile([P, 1], F32, tag="sm")
                    nc.vector.reduce_sum(out=sm[:], in_=ex[:], axis=AX.X)
                    rc = rtp.tile([P, 1], F32, tag="rc")
                    nc.vector.reciprocal(rc[:], sm[:])
                    nc.vector.tensor_scalar_mul(ex[:], ex[:], rc[:, :1])
                    nc.vector.tensor_mul(
                        comb[:, g * E : (g + 1) * E], ex[:], dcol
                    )


            # ---- mm1: h = sum_e mask_e * relu(x@W_in[e] (+ b_in)) ----
            with (
                tc.tile_pool(name="wfe", bufs=2) as wfp,
                tc.tile_pool(name="hf", bufs=2 * G) as hfp,
                tc.tile_pool(name="rtmp", bufs=4) as rtmp,
            ):
                for f in range(FC):
                    hfs = []
                    for e in range(E):
                        wfe = wfp.tile([P, KD, 512], w_store, tag="wfe")
                        nc.sync.dma_start(
                            wfe[:],
                            win_h[e, :, f * 512 : (f + 1) * 512].rearrange(
                                "(kd p) f -> p kd f", p=P
                            ),
                        )
                        if has_bin:
                            bin_sb = wfp.tile([1, 512], F32, tag="bin")
                            nc.sync.dma_start(
                                bin_sb[:],
                                bin_h[e, f * 512 : (f + 1) * 512][None, :],
                            )
                        for g in range(G):
                            ps = psp.tile([P, 512], F32, tag="ps")
                            for kd in range(KD):
                                nc.tensor.matmul(
                                    ps[:],
                                    lhsT=xTr[:, kd, g * P : (g + 1) * P],
                                    rhs=wfe[:, kd, :],
                                    start=(kd == 0),
                                    stop=(kd == KD - 1 and not has_bin),
                                )
                            if has_bin:
                                nc.tensor.matmul(
                                    ps[:],
                                    lhsT=ones1[:, :],
                                    rhs=bin_sb[:, :],
                                    start=False, stop=True,
                                )
                            sc = disp[:, g * E + e : g * E + e + 1]
                            if e == 0:
                                hf = hfp.tile([P, 512], F32, tag="hf")
                                hfs.append(hf)
                                nc.scalar.activation(
                                    hf[:], ps[:], AF.Relu, scale=sc
                                )
                            else:
                                tmp = rtmp.tile([P, 512], F32, tag="rtmp")
                                nc.scalar.activation(
                                    tmp[:], ps[:], AF.Relu, scale=sc
                                )
                                nc.vector.tensor_add(hfs[g][:], hfs[g][:], tmp[:])
                    # transpose this f-chunk of h into hT
                    for g in range(G):
                        for c in range(4):
                            pst = psp.tile([P, P], F32, tag="ps")
                            nc.tensor.transpose(
                                pst[:],
                                hfs[g][:, c * P : (c + 1) * P],
                                ident[:],
                            )
                            nc.vector.tensor_copy(
                                hT[:, f * 4 + c, g * P : (g + 1) * P], pst[:]
                            )

            # ---- mm2: y = sum_e comb_e * (h@W_out[e] (+ b_out)) ----
            ndh = 2 if wdt == F16 else 4   # D-chunk split (SBUF pressure)
            dw = D // ndh
            with tc.tile_pool(name="wo", bufs=2) as wop:
                for e in range(E):
                    for dh in range(ndh):
                        wo = wop.tile([P, FT, dw], w_store, tag="wo")
                        nc.sync.dma_start(
                            wo[:],
                            wout_h[e, :, dh * dw : (dh + 1) * dw].rearrange(
                                "(ft p) d -> p ft d", p=P
                            ),
                        )
                        if has_bout:
                            bout_sb = wop.tile([1, dw], F32, tag="bout")
                            nc.sync.dma_start(
                                bout_sb[:],
                                bout_h[e, dh * dw : (dh + 1) * dw][None, :],
                            )
                        for g in range(G):
                            ps = psp.tile([P, dw], F32, tag="ps")
                            for ft in range(FT):
                                nc.tensor.matmul(
                                    ps[:],
                                    lhsT=hT[:, ft, g * P : (g + 1) * P],
                                    rhs=wo[:, ft, :],
                                    start=(ft == 0),
                                    stop=(ft == FT - 1 and not has_bout),
                                )
                            if has_bout:
                                nc.tensor.matmul(
                                    ps[:],
                                    lhsT=ones1[:, :],
                                    rhs=bout_sb[:, :],
                                    start=False, stop=True,
                                )
                            cc = comb[:, g * E + e : g * E + e + 1]
                            ysl = yac[g][:, dh * dw : (dh + 1) * dw]
                            if e == 0:
                                nc.vector.tensor_scalar(
                                    out=ysl, in0=ps[:], scalar1=cc,
                                    scalar2=None, op0=OP.mult,
                                )
                            else:
                                tm = wop.tile([P, dw], F32, tag="ytmp")
                                nc.vector.tensor_scalar(
                                    out=tm[:], in0=ps[:], scalar1=cc,
                                    scalar2=None, op0=OP.mult,
                                )
                                nc.vector.tensor_add(ysl, ysl, tm[:])

            for g in range(G):
                nc.sync.dma_start(y_h[g * P : (g + 1) * P, :], yac[g][:])

    nc.compile()
    return nc


# ====================================================================
# V3: pair-sharded sparse kernel.
#
# Each token goes to exactly one PAIR of experts {a, b} (its top-2).
# Shard the 28 pairs across 8 cores so each core touches <= 4 distinct
# experts (two K4 halves + four 4-cycles of the K4,4 bipartite part).
# A core computes, fully locally per 128-token slab of one pair:
#     h = relu(x@W_in[a]) + relu(x@W_in[b])
#     y = p_a*(h@W_out[a]) + p_b*(h@W_out[b])
# No cross-core communication, no h spill: each expert's weights are
# read from HBM by exactly one core (the slab's expert picked from a
# resident 4-expert tile via a runtime register from a config input -
# the SPMD program is identical on all cores, only data differs).
#
# Routing (all 4096 tokens) is replicated on every core; per-pair slot
# assignment uses a strict-prefix matmul + shift-add ladder; per-slab
# payload (token row, p_a, p_b) is materialized with a one-hot
# permutation matmul (no indirect scatter on the critical path).
# x rows are gathered / y rows scattered by 4KB-row indirect DMA via a
# trash-row-0 padded x/y (padding slots read/write row 0 harmlessly).
# ====================================================================

NT = N_TOK          # 4096 tokens
GG = NT // P        # 32 token groups
NPAIR = 28
NLOC = 4            # local experts per core
PAIRS = [(a, b) for a in range(E) for b in range(a + 1, E)]
FCW = 256           # mm1 f-chunk width
NFC = F // FCW      # 16
FTL = 8             # ft-tiles per mm2 block
NFTB = FT // FTL    # 4
DW2 = 256           # mm2 d-chunk width
NDH = D // DW2      # 4


# slab -> pair-slot map shared by every core; pair-slot k gets the core's
# k-th-largest pair. The per-slot slab capacity profile is derived from the
# data (pointwise max over cores) and becomes part of the compile key.


def make_v3_plan(xf, Wr, br):
    """Host-side routing statistics -> static plan + per-core config data."""
    logits = xf @ Wr + np.asarray(br, np.float32).reshape(1, E)
    order = np.argsort(-logits, axis=-1)
    top2 = np.sort(order[:, :2], axis=1)
    pid_of = {p: k for k, p in enumerate(PAIRS)}
    pid = np.array([pid_of[(a, b)] for a, b in top2])
    cnt = np.bincount(pid, minlength=NPAIR)

    # structural pair->core assignment (<=4 experts per core)
    k4a = [(0, 1), (0, 2), (0, 3), (1, 2), (1, 3), (2, 3)]
    k4b = [(4, 5), (4, 6), (4, 7), (5, 6), (5, 7), (6, 7)]
    cycles = [
        [(0, 4), (1, 4), (1, 5), (0, 5)],
        [(0, 6), (1, 6), (1, 7), (0, 7)],
        [(2, 4), (3, 4), (3, 5), (2, 5)],
        [(2, 6), (3, 6), (3, 7), (2, 7)],
    ]
    import itertools

    def load(ps):
        return sum(int(cnt[pid_of[p]]) for p in ps)

    def best_split(edges):
        best = None
        for sub in itertools.combinations(edges, 3):
            rest = [p for p in edges if p not in sub]
            m = max(load(sub), load(rest))
            if best is None or m < best[0]:
                best = (m, list(sub), rest)
        return best[1], best[2]

    a1, a2 = best_split(k4a)
    b1, b2 = best_split(k4b)
    core_pairs = [a1, a2, b1, b2] + cycles

    sorted_pairs = []
    for c in range(NCORES):
        pairs_c = sorted(core_pairs[c], key=lambda p: -cnt[pid_of[p]])
        while len(pairs_c) < 4:
            pairs_c.append(None)
        sorted_pairs.append(pairs_c)
    ps_cap = [
        max(
            int(np.ceil(cnt[pid_of[sorted_pairs[c][j]]] / P))
            if sorted_pairs[c][j] is not None else 1
            for c in range(NCORES)
        )
        for j in range(4)
    ]
    slab_ps = [j for j in range(4) for _ in range(ps_cap[j])]

    plan = dict(nslab=len(slab_ps), slab_ps=tuple(slab_ps), cores=[])
    for c in range(NCORES):
        pairs_c = sorted_pairs[c]
        base28 = np.full((NPAIR,), -1e9, np.float32)
        s = 0
        for psi, p in enumerate(pairs_c):
            if p is not None:
                base28[pid_of[p]] = s * P
            s += ps_cap[psi]
        plan["cores"].append(dict(pairs=pairs_c, base28=base28))
    return plan


def build_nc_v3(cfg):
    nslab = cfg["nslab"]
    slab_ps = cfg["slab_ps"]
    phases = cfg.get("phases", "all")  # 'route' | 'mm1' | 'all'
    nc = bacc.Bacc(None)
    NU = 8  # pair-slot-role weight units (4 pair-slots x 2 roles)
    xp_h = nc.declare_dram_parameter("xp", [NT + 1, D], F32, isOutput=False)
    wr_h = nc.declare_dram_parameter("wr", [D, E], F32, isOutput=False)
    # host-pretiled fp16 weights stacked per pair-slot-role unit:
    #   wi[fc, p, u*kd*FCW], wo[ftb, dh, p, u*ftl*DW2]
    wi_h = nc.declare_dram_parameter(
        "wi", [NFC, P, NU * KD * FCW], F16, isOutput=False
    )
    wo_h = nc.declare_dram_parameter(
        "wo", [NFTB, NDH, P, NU * FTL * DW2], F16, isOutput=False
    )
    b28_h = nc.declare_dram_parameter("b28", [1, NPAIR], F32, isOutput=False)
    yp_h = nc.declare_dram_parameter("yp", [NT + 1, D], F32, isOutput=True)

    with tile.TileContext(nc) as tc:
        with tc.tile_pool(name="persist", bufs=1) as pp:
            # shared psum pool for router/payload/mm1; closed before mm2 so
            # mm2 can hold 7 banks of long-lived accumulators
            ps_ctx = tc.tile_pool(name="ps", bufs=8, space="PSUM")
            psp = ps_ctx.__enter__()
            from concourse.masks import make_identity, make_upper_triangular

            ident = pp.tile([P, P], F32, tag="ident")
            make_identity(nc, ident[:])
            triu = pp.tile([P, P], F32, tag="triu")
            make_upper_triangular(nc, triu[:], val=1.0, diag=False)
            # rowio[p, m] = m
            rowio_i = pp.tile([P, P], mybir.dt.int32, tag="rowio_i")
            nc.gpsimd.iota(rowio_i[:], pattern=[[1, P]], base=0,
                           channel_multiplier=0)
            rowio = pp.tile([P, P], F32, tag="rowio")
            nc.vector.tensor_copy(rowio[:], rowio_i[:])
            # nplus1[p, gg] = 1 + p + 128*gg  (token row in x_pad)
            np1_i = pp.tile([P, GG], mybir.dt.int32, tag="np1_i")
            nc.gpsimd.iota(np1_i[:], pattern=[[P, GG]], base=1,
                           channel_multiplier=1)
            np1 = pp.tile([P, GG], F32, tag="np1")
            nc.vector.tensor_copy(np1[:], np1_i[:])

            wr_sb = pp.tile([P, KD, E], F32, tag="wr")
            nc.sync.dma_start(
                wr_sb[:], wr_h[:, :].rearrange("(kd p) e -> p kd e", p=P)
            )
            b28_sb = pp.tile([1, NPAIR], F32, tag="b28")
            nc.sync.dma_start(b28_sb[:], b28_h[:])

            # routing scratch lives only until payloads are built
            rts = tc.tile_pool(name="rts", bufs=1)
            rtsp = rts.__enter__()
            disp_all = rtsp.tile([P, GG, E], F32, tag="disp_all")
            comb_all = rtsp.tile([P, GG, E], F32, tag="comb_all")

            # ---- router over all 4096 tokens ----
            with tc.tile_pool(name="rt", bufs=3) as rtp:
                for gg in range(GG):
                    xg = rtp.tile([P, D], F32, tag="xg")
                    nc.sync.dma_start(
                        xg[:], xp_h[1 + gg * P : 1 + (gg + 1) * P, :]
                    )
                    xTg = rtp.tile([P, KD, P], F32, tag="xTg")
                    for kd in range(KD):
                        pst = psp.tile([P, P], F32, tag="ps")
                        nc.tensor.transpose(
                            pst[:], xg[:, kd * P : (kd + 1) * P], ident[:]
                        )
                        nc.vector.tensor_copy(xTg[:, kd, :], pst[:])
                    psr = psp.tile([P, E], F32, tag="ps")
                    for kd in range(KD):
                        nc.tensor.matmul(
                            psr[:], lhsT=xTg[:, kd, :], rhs=wr_sb[:, kd, :],
                            start=(kd == 0), stop=(kd == KD - 1),
                        )
                    lg = rtp.tile([P, E], F32, tag="lg")
                    nc.vector.tensor_copy(lg[:], psr[:])
                    mx1 = rtp.tile([P, 1], F32, tag="mx1")
                    nmx = rtp.tile([P, 1], F32, tag="nmx")
                    nc.vector.reduce_max(out=mx1[:], in_=lg[:], axis=AX.X)
                    nc.vector.reduce_max(out=nmx[:], in_=lg[:], axis=AX.X,
                                         negate=True)
                    is1 = rtp.tile([P, E], F32, tag="is1")
                    nc.vector.tensor_scalar(
                        out=is1[:], in0=lg[:], scalar1=mx1[:, :1],
                        scalar2=None, op0=OP.is_equal,
                    )
                    nc.vector.tensor_scalar_mul(is1[:], is1[:], 1e30)
                    lgm = rtp.tile([P, E], F32, tag="lgm")
                    nc.vector.tensor_sub(lgm[:], lg[:], is1[:])
                    mx2 = rtp.tile([P, 1], F32, tag="mx2")
                    nc.vector.reduce_max(out=mx2[:], in_=lgm[:], axis=AX.X)
                    nc.vector.tensor_scalar(
                        out=disp_all[:, gg, :], in0=lg[:], scalar1=mx2[:, :1],
                        scalar2=None, op0=OP.is_ge,
                    )
                    ex = rtp.tile([P, E], F32, tag="ex")
                    nc.scalar.activation(ex[:], lg[:], AF.Exp, bias=nmx[:, :1])
                    sm = rtp.tile([P, 1], F32, tag="sm")
                    nc.vector.reduce_sum(out=sm[:], in_=ex[:], axis=AX.X)
                    rc = rtp.tile([P, 1], F32, tag="rc")
                    nc.vector.reciprocal(rc[:], sm[:])
                    nc.vector.tensor_scalar_mul(ex[:], ex[:], rc[:, :1])
                    nc.vector.tensor_mul(
                        comb_all[:, gg, :], ex[:], disp_all[:, gg, :]
                    )

            # ---- pair masks, ranks, slots, payload data ----
            # broadcast b28 across partitions via a K=1 ones matmul
            ones_r = rtsp.tile([1, P], F32, tag="ones_r")
            nc.vector.memset(ones_r[:], 1.0)
            b28_ps = psp.tile([P, NPAIR], F32, tag="ps")
            nc.tensor.matmul(b28_ps[:], lhsT=ones_r[:, :], rhs=b28_sb[:, :],
                             start=True, stop=True)
            b28_bc = rtsp.tile([P, NPAIR], F32, tag="b28_bc")
            nc.vector.tensor_copy(b28_bc[:], b28_ps[:])
            mask_all = rtsp.tile([P, NPAIR, GG], F32, tag="mask_all")
            for k, (a, b) in enumerate(PAIRS):
                nc.vector.tensor_mul(
                    mask_all[:, k, :], disp_all[:, :, a], disp_all[:, :, b]
                )
            rowsum = rtsp.tile([P, NPAIR], F32, tag="rowsum")
            nc.vector.reduce_sum(out=rowsum[:], in_=mask_all[:], axis=AX.X)
            trip_ps = psp.tile([P, NPAIR], F32, tag="ps")
            nc.tensor.matmul(trip_ps[:], lhsT=triu[:], rhs=rowsum[:],
                             start=True, stop=True)
            trip = rtsp.tile([P, NPAIR], F32, tag="trip")
            nc.vector.tensor_copy(trip[:], trip_ps[:])
            # inclusive shift-add ladder over gg, then make exclusive
            pfx_a = rtsp.tile([P, NPAIR, GG], F32, tag="pfx_a")
            pfx_b = rtsp.tile([P, NPAIR, GG], F32, tag="pfx_b")
            nc.vector.tensor_copy(pfx_a[:], mask_all[:])
            src, dst = pfx_a, pfx_b
            sh = 1
            while sh < GG:
                nc.vector.tensor_copy(dst[:, :, :sh], src[:, :, :sh])
                nc.vector.tensor_add(
                    dst[:, :, sh:], src[:, :, sh:], src[:, :, : GG - sh]
                )
                src, dst = dst, src
                sh *= 2
            # exclusive within-row prefix
            nc.vector.tensor_sub(src[:], src[:], mask_all[:])

            slot = rtsp.tile([P, GG], F32, tag="slot")
            nc.vector.memset(slot[:], 0.0)
            plo = rtsp.tile([P, GG], F32, tag="plo")
            phi = rtsp.tile([P, GG], F32, tag="phi")
            nc.vector.memset(plo[:], 0.0)
            nc.vector.memset(phi[:], 0.0)
            tmpg = rtsp.tile([P, GG], F32, tag="tmpg")
            for k, (a, b) in enumerate(PAIRS):
                # rank + base for this pair
                nc.vector.tensor_scalar(
                    out=tmpg[:], in0=src[:, k, :], scalar1=trip[:, k : k + 1],
                    scalar2=b28_bc[:, k : k + 1], op0=OP.add, op1=OP.add,
                )
                nc.vector.tensor_mul(tmpg[:], tmpg[:], mask_all[:, k, :])
                nc.vector.tensor_add(slot[:], slot[:], tmpg[:])
                nc.vector.tensor_mul(tmpg[:], mask_all[:, k, :],
                                     comb_all[:, :, a])
                nc.vector.tensor_add(plo[:], plo[:], tmpg[:])
                nc.vector.tensor_mul(tmpg[:], mask_all[:, k, :],
                                     comb_all[:, :, b])
                nc.vector.tensor_add(phi[:], phi[:], tmpg[:])

            data_all = rtsp.tile([P, GG, 4], F32, tag="data_all")
            nc.vector.memset(data_all[:], 0.0)
            nc.vector.tensor_copy(data_all[:, :, 0], np1[:])
            nc.vector.tensor_copy(data_all[:, :, 1], plo[:])
            nc.vector.tensor_copy(data_all[:, :, 2], phi[:])

            # ---- per-slab payload via one-hot permutation matmul ----
            pay = [
                pp.tile([P, 4], F32, tag=f"pay{s}", name=f"pay{s}")
                for s in range(nslab)
            ]
            idx_t = [
                pp.tile([P, 1], mybir.dt.int32, tag=f"idx{s}", name=f"idx{s}")
                for s in range(nslab)
            ]
            with tc.tile_pool(name="perm", bufs=4) as pmp:
                for s in range(nslab):
                    psq = psp.tile([P, 4], F32, tag="ps")
                    for cc in range(GG):
                        smb = pmp.tile([P, 1], F32, tag="smb")
                        nc.vector.tensor_scalar(
                            out=smb[:], in0=slot[:, cc : cc + 1],
                            scalar1=float(s * P), scalar2=None,
                            op0=OP.subtract,
                        )
                        pm = pmp.tile([P, P], F32, tag="pm")
                        nc.vector.tensor_tensor(
                            out=pm[:], in0=smb[:, :1].to_broadcast([P, P]),
                            in1=rowio[:], op=OP.is_equal,
                        )
                        nc.tensor.matmul(
                            psq[:], lhsT=pm[:], rhs=data_all[:, cc, :],
                            start=(cc == 0), stop=(cc == GG - 1),
                        )
                    nc.vector.tensor_copy(pay[s][:], psq[:])
                    nc.vector.tensor_copy(idx_t[s][:], pay[s][:, 0:1])
            rts.__exit__(None, None, None)

            if phases == "route":
                for s in range(nslab):
                    nc.sync.dma_start(yp_h[s * P : (s + 1) * P, :4], pay[s][:])
                return nc

            # ---- gather x rows, transpose per slab ----
            xTr_s = [
                pp.tile([P, KD, P], F16, tag=f"xTr{s}", name=f"xTr{s}")
                for s in range(nslab)
            ]
            with tc.tile_pool(name="gx", bufs=3) as gxp:
                for s in range(nslab):
                    xsel = gxp.tile([P, D], F32, tag="xsel")
                    nc.gpsimd.indirect_dma_start(
                        out=xsel[:], out_offset=None, in_=xp_h[:],
                        in_offset=bass.IndirectOffsetOnAxis(
                            ap=idx_t[s][:, :1], axis=0
                        ),
                    )
                    for kd in range(KD):
                        pst = psp.tile([P, P], F32, tag="ps")
                        nc.tensor.transpose(
                            pst[:], xsel[:, kd * P : (kd + 1) * P], ident[:]
                        )
                        nc.vector.tensor_copy(xTr_s[s][:, kd, :], pst[:])

            # ---- mm1 + transpose to hT ----
            hT = pp.tile([P, nslab, FT, P], F16, tag="hT")
            with (
                tc.tile_pool(name="wi", bufs=2) as wip,
                tc.tile_pool(name="hf", bufs=4) as hfp,
                tc.tile_pool(name="rt1", bufs=3) as rt1,
            ):
                # transposes of slab s's hf are emitted after slab s+1's
                # matmuls so the PE stream doesn't wait on ACT/DVE
                pending = []

                def flush_pending():
                    for hf_t, s_t, fc_t in pending:
                        for c in range(FCW // P):
                            pst = psp.tile([P, P], F32, tag="ps",
                                           name="pst_tr")
                            nc.tensor.transpose(
                                pst[:], hf_t[:, c * P : (c + 1) * P],
                                ident[:],
                            )
                            nc.vector.tensor_copy(
                                hT[:, s_t, fc_t * (FCW // P) + c, :], pst[:]
                            )
                    pending.clear()

                for fc in range(NFC):
                    wi4 = wip.tile([P, NU, KD, FCW], F16, tag="wi4")
                    wi_src = wi_h[fc].rearrange("p (u kd f) -> p u kd f",
                                                u=NU, kd=KD)
                    for q in range(4):
                        nc.sync.dma_start(
                            wi4[:, q * 2 : (q + 1) * 2],
                            wi_src[:, q * 2 : (q + 1) * 2],
                        )
                    for s in range(nslab):
                        u0 = slab_ps[s] * 2
                        ps_lo = psp.tile([P, FCW], F32, tag="ps")
                        ps_hi = psp.tile([P, FCW], F32, tag="ps")
                        for kd in range(KD):
                            nc.tensor.matmul(
                                ps_lo[:], lhsT=xTr_s[s][:, kd, :],
                                rhs=wi4[:, u0, kd, :],
                                start=(kd == 0), stop=(kd == KD - 1),
                            )
                            nc.tensor.matmul(
                                ps_hi[:], lhsT=xTr_s[s][:, kd, :],
                                rhs=wi4[:, u0 + 1, kd, :],
                                start=(kd == 0), stop=(kd == KD - 1),
                            )
                        flush_pending()
                        hf = hfp.tile([P, FCW], F32, tag="hf")
                        nc.scalar.activation(hf[:], ps_lo[:], AF.Relu)
                        ht2 = rt1.tile([P, FCW], F32, tag="ht2")
                        nc.scalar.activation(ht2[:], ps_hi[:], AF.Relu)
                        nc.vector.tensor_add(hf[:], hf[:], ht2[:])
                        pending.append((hf, s, fc))
                flush_pending()

            if phases == "mm1":
                for s in range(nslab):
                    nc.sync.dma_start(
                        yp_h[s * P : (s + 1) * P, : P // 2],
                        hT[:, s, 0, :].bitcast(F32),
                    )
                return nc

            # ---- mm2 with fused scale-accumulate flush ----
            yac3 = [
                pp.tile([P, D], F32, tag=f"ya{s}", name=f"ya{s}")
                for s in range(nslab)
            ]
            with tc.tile_pool(name="wo", bufs=2) as wop:
                for ftb in range(NFTB):
                    for dh in range(NDH):
                        wo4 = wop.tile([P, NU, FTL, DW2], F16, tag="wo4")
                        wo_src = wo_h[ftb, dh].rearrange(
                            "p (u ft d) -> p u ft d", u=NU, ft=FTL
                        )
                        for q in range(4):
                            nc.sync.dma_start(
                                wo4[:, q * 2 : (q + 1) * 2],
                                wo_src[:, q * 2 : (q + 1) * 2],
                            )
                        for s in range(nslab):
                            for r in range(2):
                                ps2 = psp.tile([P, DW2], F32, tag="ps")
                                for ftl in range(FTL):
                                    nc.tensor.matmul(
                                        ps2[:],
                                        lhsT=hT[:, s, ftb * FTL + ftl, :],
                                        rhs=wo4[:, slab_ps[s] * 2 + r, ftl, :],
                                        start=(ftl == 0), stop=(ftl == FTL - 1),
                                    )
                                ysl = yac3[s][:, dh * DW2 : (dh + 1) * DW2]
                                if ftb == 0:
                                    nc.vector.tensor_scalar(
                                        out=ysl, in0=ps2[:],
                                        scalar1=pay[s][:, 1 + r : 2 + r],
                                        scalar2=None, op0=OP.mult,
                                    ) if r == 0 else nc.vector.scalar_tensor_tensor(
                                        out=ysl, in0=ps2[:],
                                        scalar=pay[s][:, 1 + r : 2 + r],
                                        in1=ysl, op0=OP.mult, op1=OP.add,
                                    )
                                else:
                                    nc.vector.scalar_tensor_tensor(
                                        out=ysl, in0=ps2[:],
                                        scalar=pay[s][:, 1 + r : 2 + r],
                                        in1=ysl, op0=OP.mult, op1=OP.add,
                                    )

            # ---- scatter y rows ----
            for s in range(nslab):
                nc.gpsimd.indirect_dma_start(
                    out=yp_h[:],
                    out_offset=bass.IndirectOffsetOnAxis(
                        ap=idx_t[s][:, :1], axis=0
                    ),
                    in_=yac3[s][:], in_offset=None,
                )
            ps_ctx.__exit__(None, None, None)

    nc.compile()
    return nc


def make_in_maps_v3(x, Wr, br, W_in, b_in, W_out, b_out):
    xf = np.ascontiguousarray(np.asarray(x, np.float32).reshape(NT, D))
    Wr = np.asarray(Wr, np.float32)
    br = np.asarray(br, np.float32)
    assert not np.any(np.asarray(b_in)), "v3 assumes zero b_in"
    assert not np.any(np.asarray(b_out)), "v3 assumes zero b_out"
    plan = make_v3_plan(xf, Wr, br)
    nslab = plan["nslab"]
    slab_ps = plan["slab_ps"]
    x_pad = np.zeros((NT + 1, D), np.float32)
    x_pad[1:] = xf
    W_in16 = np.asarray(W_in, np.float16)
    W_out16 = np.asarray(W_out, np.float16)
    NU = 8
    in_maps = []
    for c in range(NCORES):
        pc = plan["cores"][c]
        # weight unit u = pair-slot*2 + role -> that pair's (lo, hi) expert
        unit_experts = []
        for p in pc["pairs"]:
            if p is None:
                unit_experts += [0, 0]
            else:
                unit_experts += [p[0], p[1]]
        wl_in = W_in16[unit_experts]     # [8, D, F]
        wl_out = W_out16[unit_experts]   # [8, F, D]
        wi = np.ascontiguousarray(
            wl_in.reshape(NU, KD, P, NFC, FCW)
            .transpose(3, 2, 0, 1, 4)
            .reshape(NFC, P, NU * KD * FCW)
        )
        wo = np.ascontiguousarray(
            wl_out.reshape(NU, NFTB, FTL, P, NDH, DW2)
            .transpose(1, 4, 3, 0, 2, 5)
            .reshape(NFTB, NDH, P, NU * FTL * DW2)
        )
        in_maps.append({
            "xp": x_pad,
            "wr": Wr,
            "wi": wi,
            "wo": wo,
            "b28": pc["base28"].reshape(1, NPAIR),
        })
    return slab_ps, in_maps


_NC_CACHE = {}


def get_nc(cfg_key):
    if cfg_key not in _NC_CACHE:
        cfg = dict(
            wdt=cfg_key[0], has_br=cfg_key[1], has_bin=cfg_key[2],
            has_bout=cfg_key[3],
        )
        _NC_CACHE[cfg_key] = build_nc(cfg)
    return _NC_CACHE[cfg_key]


WDT_MODE = os.environ.get("MOE_WDT", "f32r")


def make_in_maps(x, Wr, br, W_in, b_in, W_out, b_out, wdt_mode):
    xf = np.ascontiguousarray(np.asarray(x, np.float32).reshape(N_TOK, D))
    w_store_np = np.float32 if wdt_mode == "f32r" else np.float16
    win = np.ascontiguousarray(np.asarray(W_in, w_store_np))
    wout = np.ascontiguousarray(np.asarray(W_out, w_store_np))
    wr = np.ascontiguousarray(np.asarray(Wr, np.float32))
    has_br = bool(np.any(np.asarray(br) != 0))
    has_bin = bool(np.any(np.asarray(b_in) != 0))
    has_bout = bool(np.any(np.asarray(b_out) != 0))
    in_maps = []
    for c in range(NCORES):
        m = {
            "x": xf[c * T : (c + 1) * T],
            "wr": wr,
            "w_in": win,
            "w_out": wout,
        }
        if has_br:
            m["br"] = np.asarray(br, np.float32).reshape(1, E)
        if has_bin:
            m["b_in"] = np.asarray(b_in, np.float32)
        if has_bout:
            m["b_out"] = np.asarray(b_out, np.float32)
        in_maps.append(m)
    cfg_key = (wdt_mode, has_br, has_bin, has_bout)
    return cfg_key, in_maps


def get_nc_v3(slab_ps):
    key = ("v3", tuple(slab_ps))
    if key not in _NC_CACHE:
        _NC_CACHE[key] = build_nc_v3(
            dict(nslab=len(slab_ps), slab_ps=tuple(slab_ps))
        )
    return _NC_CACHE[key]


# v3 = pair-sharded sparse (default); v1 = data-parallel dense fallback
# (v1 also serves as the general path when any bias is nonzero)
IMPL = os.environ.get("MOE_IMPL", "v3")


def kernel(x, Wr, br, W_in, b_in, W_out, b_out, top_k):
    assert int(top_k) == 2, "kernel is specialized for top_k=2"
    if IMPL == "v3" and not (np.any(np.asarray(b_in)) or np.any(np.asarray(b_out)) or np.any(np.asarray(br))):
        slab_ps, in_maps = make_in_maps_v3(x, Wr, br, W_in, b_in, W_out, b_out)
        nc = get_nc_v3(slab_ps)
        res = run_bass_kernel_spmd(nc, in_maps, list(range(NCORES)))
        y = np.zeros((NT, D), np.float32)
        for c in range(NCORES):
            y += res.results[c]["yp"][1:]
        return y.reshape(4, 1024, 1024)
    cfg_key, in_maps = make_in_maps(
        x, Wr, br, W_in, b_in, W_out, b_out, WDT_MODE
    )
    nc = get_nc(cfg_key)
    res = run_bass_kernel_spmd(nc, in_maps, list(range(NCORES)))
    y = np.concatenate([res.results[c]["y"] for c in range(NCORES)], axis=0)
    return y.reshape(4, 1024, 1024).astype(np.float32)



# revision 4
# speedup vs baseline: 3.8625x; 3.8625x over previous
"""MoE feed-forward (top-2 of 8 experts) Trainium2 Bass kernel.

Problem: nn_MixtureOfExpertsFeedForward_6734508720763
  x[4,1024,1024] tokens, router Wr[1024,8], experts W_in[8,1024,4096],
  W_out[8,4096,1024], top_k=2.

  ref:  logits = x@Wr + br ; probs = softmax(logits)
        top2 -> dispatch (0/1), combine (prob or 0)
        h = sum_e dispatch[n,e] * relu(x @ W_in[e] + b_in[e])
        y = sum_e combine[n,e]  * (h @ W_out[e] + b_out[e])

V4 strategy (expert parallelism, host-side all-to-all dispatch):
  Core e owns expert e. The host computes the (tiny, 67 MFLOP) router,
  gathers each expert's routed tokens, pre-scales each token row by its
  combine prob p (valid since p>0: p*relu(z) == relu(p*z) and the output
  Linear is linear), pads every expert to a common CAP so the SPMD
  program is shape-identical, and scatter-adds the per-expert outputs.

  The device program per core is a dense relu(x @ W_in) @ W_out with the
  WEIGHTS as the stationary matmul operand and the tokens as the moving
  (free) axis:
    mm1:  hT[ftile, t] += W_in[kd, ftile].T @ xT[kd, t]   (accum over kd)
    mm2:  yT[dtile, t] += W_out[ftc, dtile].T @ hT[ftc, t] (accum over ftc)
  so mm1's output is already transposed for mm2 -> ZERO PE transposes and
  the PE stream is nothing but back-to-back fp16 matmuls. x is gathered /
  transposed / fp16-cast on the host; weights are host-pretiled so every
  DMA row is >=2KB contiguous.

V1 fallback (dense over experts, data parallel) retained for nonzero
b_in/b_out inputs.
"""

import os
import sys

import numpy as np

sys.path.insert(0, "/opt/trn_rl_repo")

import concourse.bacc as bacc
import concourse.bass as bass
import concourse.mybir as mybir
import concourse.tile as tile
from concourse.bass_utils import run_bass_kernel_spmd

F32 = mybir.dt.float32
F32R = mybir.dt.float32r
F16 = mybir.dt.float16

P = 128          # partitions
NCORES = 8
N_TOK = 4096     # total tokens (4*1024)
T = N_TOK // NCORES   # tokens per core = 512 (v1 path)
G = T // P       # token groups per core = 4 (v1 path)
D = 1024
KD = D // P      # 8 contraction chunks for D
F = 4096
FC = F // 512    # 8 f-chunks of 512 (v1 path)
FT = F // P      # 32 f-tiles of 128
DT = D // P      # 8 d-tiles of 128
E = 8
NT = N_TOK
AX = mybir.AxisListType
AF = mybir.ActivationFunctionType
OP = mybir.AluOpType


# ====================================================================
# V4: expert-parallel, host-dispatched, transpose-free.
# ====================================================================


def _chunks(cap):
    """Split cap token columns into <=512-wide PSUM-bank chunks."""
    nch = -(-cap // 512)
    sizes = [cap // nch + (1 if i < cap % nch else 0) for i in range(nch)]
    offs = [0]
    for s in sizes:
        offs.append(offs[-1] + s)
    return nch, sizes, offs


def build_nc_v4(cap):
    nch, sizes, offs = _chunks(cap)
    nc = bacc.Bacc(None)
    xT_h = nc.declare_dram_parameter("xT", [D, cap], F16, isOutput=False)
    wi_h = nc.declare_dram_parameter("wi", [FT, P, KD * P], F16, isOutput=False)
    wo_h = nc.declare_dram_parameter("wo", [DT, P, FT * P], F16, isOutput=False)
    yt_h = nc.declare_dram_parameter("yt", [D, cap], F16, isOutput=True)

    with tile.TileContext(nc) as tc:
        with (
            tc.tile_pool(name="persist", bufs=1) as pp,
            tc.tile_pool(name="ps", bufs=8, space="PSUM") as psp,
            tc.tile_pool(name="wi", bufs=3) as wip,
            tc.tile_pool(name="wo", bufs=2) as wop,
            tc.tile_pool(name="yt", bufs=2) as ytp,
        ):
            xT = pp.tile([P, KD, cap], F16, tag="xT")
            hT = pp.tile([P, FT, cap], F16, tag="hT")
            for kd in range(KD):
                nc.sync.dma_start(xT[:, kd, :], xT_h[kd * P : (kd + 1) * P, :])

            # mm1: hT[ft, t] = relu(sum_kd W_in[kd, ft].T @ xT[kd, t])
            for ft in range(FT):
                wi_sb = wip.tile([P, KD, P], F16, tag="wi")
                nc.sync.dma_start(
                    wi_sb[:], wi_h[ft].rearrange("p (kd f) -> p kd f", kd=KD)
                )
                pss = [
                    psp.tile([P, 512], F32, tag="ps", name=f"ps1_{ch}")
                    for ch in range(nch)
                ]
                for kd in range(KD):
                    for ch in range(nch):
                        o, w = offs[ch], sizes[ch]
                        nc.tensor.matmul(
                            pss[ch][:, :w],
                            lhsT=wi_sb[:, kd, :],
                            rhs=xT[:, kd, o : o + w],
                            start=(kd == 0),
                            stop=(kd == KD - 1),
                        )
                for ch in range(nch):
                    o, w = offs[ch], sizes[ch]
                    nc.scalar.activation(
                        hT[:, ft, o : o + w], pss[ch][:, :w], AF.Relu
                    )

            # mm2: yT[dt, t] = sum_ftc W_out[ftc, dt].T @ hT[ftc, t]
            for dt in range(DT):
                wo_sb = wop.tile([P, FT, P], F16, tag="wo")
                nc.sync.dma_start(
                    wo_sb[:], wo_h[dt].rearrange("p (ftc d) -> p ftc d", ftc=FT)
                )
                ps2 = [
                    psp.tile([P, 512], F32, tag="ps", name=f"ps2_{ch}")
                    for ch in range(nch)
                ]
                for ftc in range(FT):
                    for ch in range(nch):
                        o, w = offs[ch], sizes[ch]
                        nc.tensor.matmul(
                            ps2[ch][:, :w],
                            lhsT=wo_sb[:, ftc, :],
                            rhs=hT[:, ftc, o : o + w],
                            start=(ftc == 0),
                            stop=(ftc == FT - 1),
                        )
                yt = ytp.tile([P, cap], F16, tag="yt")
                for ch in range(nch):
                    o, w = offs[ch], sizes[ch]
                    nc.vector.tensor_copy(yt[:, o : o + w], ps2[ch][:, :w])
                nc.sync.dma_start(yt_h[dt * P : (dt + 1) * P, :], yt[:])

    nc.compile()
    return nc


def route_v4(xf, Wr, br):
    """Host router: per-expert token index lists + combine probs."""
    logits = xf @ np.asarray(Wr, np.float32) + np.asarray(
        br, np.float32
    ).reshape(1, E)
    order = np.argsort(-logits, axis=-1, kind="stable")
    top2 = order[:, :2]
    mx = logits.max(axis=-1, keepdims=True)
    ex = np.exp(logits - mx)
    probs = ex / ex.sum(axis=-1, keepdims=True)
    idx_list, p_list = [], []
    for e in range(E):
        sel = np.nonzero((top2 == e).any(axis=1))[0]
        idx_list.append(sel)
        p_list.append(probs[sel, e].astype(np.float32))
    cap = max(1, max(len(s) for s in idx_list))
    cap = -(-cap // 16) * 16
    return idx_list, p_list, cap


def make_in_maps_v4(x, W_in, W_out, idx_list, p_list, cap):
    xf = np.asarray(x, np.float32).reshape(NT, D)
    in_maps = []
    for e in range(E):
        sel = idx_list[e]
        xs = np.zeros((cap, D), np.float32)
        xs[: len(sel)] = xf[sel] * p_list[e][:, None]
        xT = np.ascontiguousarray(xs.T.astype(np.float16))
        wi = np.ascontiguousarray(
            np.asarray(W_in[e], np.float16)
            .reshape(KD, P, FT, P)
            .transpose(2, 1, 0, 3)
        ).reshape(FT, P, KD * P)
        wo = np.ascontiguousarray(
            np.asarray(W_out[e], np.float16)
            .reshape(FT, P, DT, P)
            .transpose(2, 1, 0, 3)
        ).reshape(DT, P, FT * P)
        in_maps.append({"xT": xT, "wi": wi, "wo": wo})
    return in_maps


# ====================================================================
# V1: dense-over-experts data-parallel fallback (handles any biases).
# ====================================================================


def build_nc(cfg):
    """Build the single-core SPMD bass program (dense over experts).

    cfg keys: wdt ('f32r'|'f16') - dtype of expert weights + hT in matmuls;
              has_br/has_bin/has_bout - include bias adds.
    """
    wdt = F32R if cfg["wdt"] == "f32r" else F16
    w_store = F32R if cfg["wdt"] == "f32r" else F16
    has_br = cfg["has_br"]
    has_bin = cfg["has_bin"]
    has_bout = cfg["has_bout"]

    nc = bacc.Bacc(None)
    x_h = nc.declare_dram_parameter("x", [T, D], F32, isOutput=False)
    wr_h = nc.declare_dram_parameter("wr", [D, E], F32, isOutput=False)
    win_h = nc.declare_dram_parameter("w_in", [E, D, F], w_store, isOutput=False)
    wout_h = nc.declare_dram_parameter("w_out", [E, F, D], w_store, isOutput=False)
    br_h = nc.declare_dram_parameter("br", [1, E], F32, isOutput=False) if has_br else None
    bin_h = nc.declare_dram_parameter("b_in", [E, F], F32, isOutput=False) if has_bin else None
    bout_h = nc.declare_dram_parameter("b_out", [E, D], F32, isOutput=False) if has_bout else None
    y_h = nc.declare_dram_parameter("y", [T, D], F32, isOutput=True)

    with tile.TileContext(nc) as tc:
        with (
            tc.tile_pool(name="persist", bufs=1) as pp,
            tc.tile_pool(name="ps", bufs=6, space="PSUM") as psp,
        ):
            ident = pp.tile([P, P], F32, tag="ident")
            from concourse.masks import make_identity
            make_identity(nc, ident[:])

            xT = pp.tile([P, KD, T], F32, tag="xT")          # x transposed, f32
            hT = pp.tile([P, FT, T], w_store, tag="hT")      # h transposed
            xTr = pp.tile([P, KD, T], w_store, tag="xTr", name="xTr")
            wr_sb = pp.tile([P, KD, E], F32, tag="wr")
            disp = pp.tile([P, G * E], F32, tag="disp")      # dispatch mask
            comb = pp.tile([P, G * E], F32, tag="comb")      # combine probs
            yac = [
                pp.tile([P, D], F32, tag=f"y{g}", name=f"yac{g}")
                for g in range(G)
            ]
            ones1 = pp.tile([1, P], F32, tag="ones1")
            if has_bin or has_bout:
                nc.vector.memset(ones1[:], 1.0)
            br_sb = None
            if has_br:
                br_sb = pp.tile([1, E], F32, tag="br")
                nc.sync.dma_start(br_sb[:], br_h[:])

            nc.sync.dma_start(
                wr_sb[:], wr_h[:, :].rearrange("(kd p) e -> p kd e", p=P)
            )

            with tc.tile_pool(name="xload", bufs=2) as xlp:
                for g in range(G):
                    xg = xlp.tile([P, D], F32, tag="xg")
                    nc.sync.dma_start(xg[:], x_h[g * P : (g + 1) * P, :])
                    for kd in range(KD):
                        pst = psp.tile([P, P], F32, tag="ps")
                        nc.tensor.transpose(
                            pst[:], xg[:, kd * P : (kd + 1) * P], ident[:]
                        )
                        nc.vector.tensor_copy(
                            xT[:, kd, g * P : (g + 1) * P], pst[:]
                        )
                        nc.vector.tensor_copy(
                            xTr[:, kd, g * P : (g + 1) * P], pst[:]
                        )

            # router (true fp32 matmul; top-2 must match reference)
            with tc.tile_pool(name="rt", bufs=2) as rtp:
                for g in range(G):
                    psr = psp.tile([P, E], F32, tag="ps")
                    for kd in range(KD):
                        nc.tensor.matmul(
                            psr[:],
                            lhsT=xT[:, kd, g * P : (g + 1) * P],
                            rhs=wr_sb[:, kd, :],
                            start=(kd == 0),
                            stop=(kd == KD - 1 and not has_br),
                        )
                    if has_br:
                        nc.tensor.matmul(
                            psr[:], lhsT=ones1[:, :], rhs=br_sb[:, :],
                            start=False, stop=True,
                        )
                    lg = rtp.tile([P, E], F32, tag="lg")
                    nc.vector.tensor_copy(lg[:], psr[:])
                    mx1 = rtp.tile([P, 1], F32, tag="mx1")
                    nmx = rtp.tile([P, 1], F32, tag="nmx")
                    nc.vector.reduce_max(out=mx1[:], in_=lg[:], axis=AX.X)
                    nc.vector.reduce_max(out=nmx[:], in_=lg[:], axis=AX.X, negate=True)
                    is1 = rtp.tile([P, E], F32, tag="is1")
                    nc.vector.tensor_scalar(
                        out=is1[:], in0=lg[:], scalar1=mx1[:, :1], scalar2=None,
                        op0=OP.is_equal,
                    )
                    lgm = rtp.tile([P, E], F32, tag="lgm")
                    nc.vector.tensor_scalar_mul(is1[:], is1[:], 1e30)
                    nc.vector.tensor_sub(lgm[:], lg[:], is1[:])
                    mx2 = rtp.tile([P, 1], F32, tag="mx2")
                    nc.vector.reduce_max(out=mx2[:], in_=lgm[:], axis=AX.X)
                    dcol = disp[:, g * E : (g + 1) * E]
                    nc.vector.tensor_scalar(
                        out=dcol, in0=lg[:], scalar1=mx2[:, :1], scalar2=None,
                        op0=OP.is_ge,
                    )
                    ex = rtp.tile([P, E], F32, tag="ex")
                    nc.scalar.activation(ex[:], lg[:], AF.Exp, bias=nmx[:, :1])
                    sm = rtp.tile([P, 1], F32, tag="sm")
                    nc.vector.reduce_sum(out=sm[:], in_=ex[:], axis=AX.X)
                    rc = rtp.tile([P, 1], F32, tag="rc")
                    nc.vector.reciprocal(rc[:], sm[:])
                    nc.vector.tensor_scalar_mul(ex[:], ex[:], rc[:, :1])
                    nc.vector.tensor_mul(
                        comb[:, g * E : (g + 1) * E], ex[:], dcol
                    )

            # mm1: h = sum_e mask_e * relu(x@W_in[e] (+ b_in))
            with (
                tc.tile_pool(name="wfe", bufs=2) as wfp,
                tc.tile_pool(name="hf", bufs=2 * G) as hfp,
                tc.tile_pool(name="rtmp", bufs=4) as rtmp,
            ):
                for f in range(FC):
                    hfs = []
                    for e in range(E):
                        wfe = wfp.tile([P, KD, 512], w_store, tag="wfe")
                        nc.sync.dma_start(
                            wfe[:],
                            win_h[e, :, f * 512 : (f + 1) * 512].rearrange(
                                "(kd p) f -> p kd f", p=P
                            ),
                        )
                        if has_bin:
                            bin_sb = wfp.tile([1, 512], F32, tag="bin")
                            nc.sync.dma_start(
                                bin_sb[:],
                                bin_h[e, f * 512 : (f + 1) * 512][None, :],
                            )
                        for g in range(G):
                            ps = psp.tile([P, 512], F32, tag="ps")
                            for kd in range(KD):
                                nc.tensor.matmul(
                                    ps[:],
                                    lhsT=xTr[:, kd, g * P : (g + 1) * P],
                                    rhs=wfe[:, kd, :],
                                    start=(kd == 0),
                                    stop=(kd == KD - 1 and not has_bin),
                                )
                            if has_bin:
                                nc.tensor.matmul(
                                    ps[:],
                                    lhsT=ones1[:, :],
                                    rhs=bin_sb[:, :],
                                    start=False, stop=True,
                                )
                            sc = disp[:, g * E + e : g * E + e + 1]
                            if e == 0:
                                hf = hfp.tile([P, 512], F32, tag="hf")
                                hfs.append(hf)
                                nc.scalar.activation(
                                    hf[:], ps[:], AF.Relu, scale=sc
                                )
                            else:
                                tmp = rtmp.tile([P, 512], F32, tag="rtmp")
                                nc.scalar.activation(
                                    tmp[:], ps[:], AF.Relu, scale=sc
                                )
                                nc.vector.tensor_add(hfs[g][:], hfs[g][:], tmp[:])
                    for g in range(G):
                        for c in range(4):
                            pst = psp.tile([P, P], F32, tag="ps")
                            nc.tensor.transpose(
                                pst[:],
                                hfs[g][:, c * P : (c + 1) * P],
                                ident[:],
                            )
                            nc.vector.tensor_copy(
                                hT[:, f * 4 + c, g * P : (g + 1) * P], pst[:]
                            )

            # mm2: y = sum_e comb_e * (h@W_out[e] (+ b_out))
            ndh = 2 if wdt == F16 else 4
            dw = D // ndh
            with tc.tile_pool(name="wo", bufs=2) as wop:
                for e in range(E):
                    for dh in range(ndh):
                        wo = wop.tile([P, FT, dw], w_store, tag="wo")
                        nc.sync.dma_start(
                            wo[:],
                            wout_h[e, :, dh * dw : (dh + 1) * dw].rearrange(
                                "(ft p) d -> p ft d", p=P
                            ),
                        )
                        if has_bout:
                            bout_sb = wop.tile([1, dw], F32, tag="bout")
                            nc.sync.dma_start(
                                bout_sb[:],
                                bout_h[e, dh * dw : (dh + 1) * dw][None, :],
                            )
                        for g in range(G):
                            ps = psp.tile([P, dw], F32, tag="ps")
                            for ft in range(FT):
                                nc.tensor.matmul(
                                    ps[:],
                                    lhsT=hT[:, ft, g * P : (g + 1) * P],
                                    rhs=wo[:, ft, :],
                                    start=(ft == 0),
                                    stop=(ft == FT - 1 and not has_bout),
                                )
                            if has_bout:
                                nc.tensor.matmul(
                                    ps[:],
                                    lhsT=ones1[:, :],
                                    rhs=bout_sb[:, :],
                                    start=False, stop=True,
                                )
                            cc = comb[:, g * E + e : g * E + e + 1]
                            ysl = yac[g][:, dh * dw : (dh + 1) * dw]
                            if e == 0:
                                nc.vector.tensor_scalar(
                                    out=ysl, in0=ps[:], scalar1=cc,
                                    scalar2=None, op0=OP.mult,
                                )
                            else:
                                tm = wop.tile([P, dw], F32, tag="ytmp")
                                nc.vector.tensor_scalar(
                                    out=tm[:], in0=ps[:], scalar1=cc,
                                    scalar2=None, op0=OP.mult,
                                )
                                nc.vector.tensor_add(ysl, ysl, tm[:])

            for g in range(G):
                nc.sync.dma_start(y_h[g * P : (g + 1) * P, :], yac[g][:])

    nc.compile()
    return nc


_NC_CACHE = {}


def get_nc(cfg_key):
    if cfg_key not in _NC_CACHE:
        cfg = dict(
            wdt=cfg_key[0], has_br=cfg_key[1], has_bin=cfg_key[2],
            has_bout=cfg_key[3],
        )
        _NC_CACHE[cfg_key] = build_nc(cfg)
    return _NC_CACHE[cfg_key]


def get_nc_v4(cap):
    key = ("v4", cap)
    if key not in _NC_CACHE:
        _NC_CACHE[key] = build_nc_v4(cap)
    return _NC_CACHE[key]


WDT_MODE = os.environ.get("MOE_WDT", "f16")


def make_in_maps(x, Wr, br, W_in, b_in, W_out, b_out, wdt_mode):
    xf = np.ascontiguousarray(np.asarray(x, np.float32).reshape(N_TOK, D))
    w_store_np = np.float32 if wdt_mode == "f32r" else np.float16
    win = np.ascontiguousarray(np.asarray(W_in, w_store_np))
    wout = np.ascontiguousarray(np.asarray(W_out, w_store_np))
    wr = np.ascontiguousarray(np.asarray(Wr, np.float32))
    has_br = bool(np.any(np.asarray(br) != 0))
    has_bin = bool(np.any(np.asarray(b_in) != 0))
    has_bout = bool(np.any(np.asarray(b_out) != 0))
    in_maps = []
    for c in range(NCORES):
        m = {
            "x": xf[c * T : (c + 1) * T],
            "wr": wr,
            "w_in": win,
            "w_out": wout,
        }
        if has_br:
            m["br"] = np.asarray(br, np.float32).reshape(1, E)
        if has_bin:
            m["b_in"] = np.asarray(b_in, np.float32)
        if has_bout:
            m["b_out"] = np.asarray(b_out, np.float32)
        in_maps.append(m)
    cfg_key = (wdt_mode, has_br, has_bin, has_bout)
    return cfg_key, in_maps


# v4 = expert-parallel host-dispatched (default); v1 = dense fallback
# (v1 also serves as the general path when b_in/b_out is nonzero)
IMPL = os.environ.get("MOE_IMPL", "v4")


def kernel(x, Wr, br, W_in, b_in, W_out, b_out, top_k):
    assert int(top_k) == 2, "kernel is specialized for top_k=2"
    if IMPL == "v4" and not (np.any(np.asarray(b_in)) or np.any(np.asarray(b_out))):
        xf = np.ascontiguousarray(np.asarray(x, np.float32).reshape(NT, D))
        idx_list, p_list, cap = route_v4(xf, Wr, br)
        in_maps = make_in_maps_v4(x, W_in, W_out, idx_list, p_list, cap)
        nc = get_nc_v4(cap)
        res = run_bass_kernel_spmd(nc, in_maps, list(range(NCORES)))
        y = np.zeros((NT, D), np.float32)
        for e in range(E):
            n = len(idx_list[e])
            ye = np.asarray(res.results[e]["yt"])  # [D, cap] f16
            y[idx_list[e]] += ye[:, :n].T.astype(np.float32)
        return y.reshape(4, 1024, 1024)
    cfg_key, in_maps = make_in_maps(
        x, Wr, br, W_in, b_in, W_out, b_out, WDT_MODE
    )
    nc = get_nc(cfg_key)
    res = run_bass_kernel_spmd(nc, in_maps, list(range(NCORES)))
    y = np.concatenate([res.results[c]["y"] for c in range(NCORES)], axis=0)
    return y.reshape(4, 1024, 1024).astype(np.float32)


# revision 6
# speedup vs baseline: 4.0200x; 1.0408x over previous
"""MoE feed-forward (top-2 of 8 experts) Trainium2 Bass kernel.

Problem: nn_MixtureOfExpertsFeedForward_6734508720763
  x[4,1024,1024] tokens, router Wr[1024,8], experts W_in[8,1024,4096],
  W_out[8,4096,1024], top_k=2.

  ref:  logits = x@Wr + br ; probs = softmax(logits)
        top2 -> dispatch (0/1), combine (prob or 0)
        h = sum_e dispatch[n,e] * relu(x @ W_in[e] + b_in[e])
        y = sum_e combine[n,e]  * (h @ W_out[e] + b_out[e])

V4 strategy (expert parallelism, host-side all-to-all dispatch):
  Core e owns expert e. The host computes the (tiny, 67 MFLOP) router,
  gathers each expert's routed tokens, pre-scales each token row by its
  combine prob p (valid since p>0: p*relu(z) == relu(p*z) and the output
  Linear is linear), pads every expert to a common CAP so the SPMD
  program is shape-identical, and scatter-adds the per-expert outputs.

  The device program per core is a dense relu(x @ W_in) @ W_out with the
  WEIGHTS as the stationary matmul operand and the tokens as the moving
  (free) axis:
    mm1:  hT[ftile, t] += W_in[kd, ftile].T @ xT[kd, t]   (accum over kd)
    mm2:  yT[dtile, t] += W_out[ftc, dtile].T @ hT[ftc, t] (accum over ftc)
  so mm1's output is already transposed for mm2 -> ZERO PE transposes and
  the PE stream is nothing but back-to-back fp16 matmuls. x is gathered /
  transposed / fp16-cast on the host; weights are host-pretiled so every
  DMA row is >=2KB contiguous.

V1 fallback (dense over experts, data parallel) retained for nonzero
b_in/b_out inputs.
"""

import os
import sys

import numpy as np

sys.path.insert(0, "/opt/trn_rl_repo")

import concourse.bacc as bacc
import concourse.bass as bass
import concourse.mybir as mybir
import concourse.tile as tile
from concourse.bass_utils import run_bass_kernel_spmd

F32 = mybir.dt.float32
F32R = mybir.dt.float32r
F16 = mybir.dt.float16

P = 128          # partitions
NCORES = 8
N_TOK = 4096     # total tokens (4*1024)
T = N_TOK // NCORES   # tokens per core = 512 (v1 path)
G = T // P       # token groups per core = 4 (v1 path)
D = 1024
KD = D // P      # 8 contraction chunks for D
F = 4096
FC = F // 512    # 8 f-chunks of 512 (v1 path)
FT = F // P      # 32 f-tiles of 128
DT = D // P      # 8 d-tiles of 128
E = 8
NT = N_TOK
AX = mybir.AxisListType
AF = mybir.ActivationFunctionType
OP = mybir.AluOpType


# ====================================================================
# V4: expert-parallel, host-dispatched, transpose-free.
# ====================================================================


def _chunks(cap):
    """Split cap token columns into <=512-wide PSUM-bank chunks.

    All-but-last chunks are 512 wide; the last carries the remainder so
    the final output copy + DMA on the critical tail is small.
    """
    nch = -(-cap // 512)
    sizes = [512] * (nch - 1) + [cap - 512 * (nch - 1)]
    offs = [0]
    for s in sizes:
        offs.append(offs[-1] + s)
    return nch, sizes, offs


# PE warm-up: dependency-free matmuls bridging the initial weight/x DMA
# so the tensor engine's p-state ramp (cost model: 3us of continuous
# execution) completes before the first real matmul issues.
WARM_N = 64
WARM_COUNT = 350


def build_nc_v4(cap):
    nch, sizes, offs = _chunks(cap)
    nc = bacc.Bacc(None)
    xT_h = nc.declare_dram_parameter("xT", [D, cap], F16, isOutput=False)
    wi_h = nc.declare_dram_parameter("wi", [FT, P, KD * P], F16, isOutput=False)
    wo_h = nc.declare_dram_parameter("wo", [DT, P, FT * P], F16, isOutput=False)
    yt_h = nc.declare_dram_parameter("yt", [D, cap], F16, isOutput=True)

    with tile.TileContext(nc) as tc:
        with (
            tc.tile_pool(name="persist", bufs=1) as pp,
            tc.tile_pool(name="ps", bufs=8, space="PSUM") as psp,
            tc.tile_pool(name="wi", bufs=3) as wip,
            tc.tile_pool(name="wo", bufs=2) as wop,
            tc.tile_pool(name="yt", bufs=2) as ytp,
        ):
            xT = pp.tile([P, KD, cap], F16, tag="xT")
            hT = pp.tile([P, FT, cap], F16, tag="hT")

            # first weight tile + x, in the order mm1 consumes them
            wi_sb0 = wip.tile([P, KD, P], F16, tag="wi", name="wi0")
            nc.sync.dma_start(
                wi_sb0[:], wi_h[0].rearrange("p (kd f) -> p kd f", kd=KD)
            )
            for kd in range(KD):
                nc.sync.dma_start(xT[:, kd, :], xT_h[kd * P : (kd + 1) * P, :])

            # dependency-free PE warm-up while the DMAs land
            wsrc = pp.tile([P, WARM_N], F16, tag="wsrc")
            nc.vector.memset(wsrc[:], 0.0)
            wps = psp.tile([P, 512], F32, tag="ps", name="wps")
            for _ in range(WARM_COUNT):
                nc.tensor.matmul(
                    wps[:WARM_N, :WARM_N],
                    lhsT=wsrc[:, :],
                    rhs=wsrc[:, :],
                    start=True,
                    stop=True,
                )

            # mm1: hT[ft, t] = relu(sum_kd W_in[kd, ft].T @ xT[kd, t])
            for ft in range(FT):
                if ft == 0:
                    wi_sb = wi_sb0
                else:
                    wi_sb = wip.tile([P, KD, P], F16, tag="wi", name="wi")
                    nc.sync.dma_start(
                        wi_sb[:], wi_h[ft].rearrange("p (kd f) -> p kd f", kd=KD)
                    )
                for ch in range(nch):
                    o, w = offs[ch], sizes[ch]
                    ps = psp.tile([P, 512], F32, tag="ps", name="ps1")
                    for kd in range(KD):
                        nc.tensor.matmul(
                            ps[:, :w],
                            lhsT=wi_sb[:, kd, :],
                            rhs=xT[:, kd, o : o + w],
                            start=(kd == 0),
                            stop=(kd == KD - 1),
                        )
                    nc.scalar.activation(
                        hT[:, ft, o : o + w], ps[:, :w], AF.Relu
                    )

            # mm2: yT[dt, t] = sum_ftc W_out[ftc, dt].T @ hT[ftc, t]
            for dt in range(DT):
                wo_sb = wop.tile([P, FT, P], F16, tag="wo", name="wo")
                nc.sync.dma_start(
                    wo_sb[:], wo_h[dt].rearrange("p (ftc d) -> p ftc d", ftc=FT)
                )
                yt = ytp.tile([P, cap], F16, tag="yt", name="yt")
                split_out = dt == DT - 1
                for ch in range(nch):
                    o, w = offs[ch], sizes[ch]
                    ps = psp.tile([P, 512], F32, tag="ps", name="ps2")
                    for ftc in range(FT):
                        nc.tensor.matmul(
                            ps[:, :w],
                            lhsT=wo_sb[:, ftc, :],
                            rhs=hT[:, ftc, o : o + w],
                            start=(ftc == 0),
                            stop=(ftc == FT - 1),
                        )
                    nc.vector.tensor_copy(yt[:, o : o + w], ps[:, :w])
                    if split_out:
                        nc.sync.dma_start(
                            yt_h[dt * P : (dt + 1) * P, o : o + w],
                            yt[:, o : o + w],
                        )
                if not split_out:
                    nc.sync.dma_start(yt_h[dt * P : (dt + 1) * P, :], yt[:])

    nc.compile()
    return nc


def route_v4(xf, Wr, br):
    """Host router: per-expert token index lists + combine probs."""
    logits = xf @ np.asarray(Wr, np.float32) + np.asarray(
        br, np.float32
    ).reshape(1, E)
    order = np.argsort(-logits, axis=-1, kind="stable")
    top2 = order[:, :2]
    mx = logits.max(axis=-1, keepdims=True)
    ex = np.exp(logits - mx)
    probs = ex / ex.sum(axis=-1, keepdims=True)
    idx_list, p_list = [], []
    for e in range(E):
        sel = np.nonzero((top2 == e).any(axis=1))[0]
        idx_list.append(sel)
        p_list.append(probs[sel, e].astype(np.float32))
    cap = max(16, max(len(s) for s in idx_list))
    cap = -(-cap // 2) * 2
    return idx_list, p_list, cap


def make_in_maps_v4(x, W_in, W_out, idx_list, p_list, cap):
    xf = np.asarray(x, np.float32).reshape(NT, D)
    in_maps = []
    for e in range(E):
        sel = idx_list[e]
        xs = np.zeros((cap, D), np.float32)
        xs[: len(sel)] = xf[sel] * p_list[e][:, None]
        xT = np.ascontiguousarray(xs.T.astype(np.float16))
        wi = np.ascontiguousarray(
            np.asarray(W_in[e], np.float16)
            .reshape(KD, P, FT, P)
            .transpose(2, 1, 0, 3)
        ).reshape(FT, P, KD * P)
        wo = np.ascontiguousarray(
            np.asarray(W_out[e], np.float16)
            .reshape(FT, P, DT, P)
            .transpose(2, 1, 0, 3)
        ).reshape(DT, P, FT * P)
        in_maps.append({"xT": xT, "wi": wi, "wo": wo})
    return in_maps


# ====================================================================
# V1: dense-over-experts data-parallel fallback (handles any biases).
# ====================================================================


def build_nc(cfg):
    """Build the single-core SPMD bass program (dense over experts).

    cfg keys: wdt ('f32r'|'f16') - dtype of expert weights + hT in matmuls;
              has_br/has_bin/has_bout - include bias adds.
    """
    wdt = F32R if cfg["wdt"] == "f32r" else F16
    w_store = F32R if cfg["wdt"] == "f32r" else F16
    has_br = cfg["has_br"]
    has_bin = cfg["has_bin"]
    has_bout = cfg["has_bout"]

    nc = bacc.Bacc(None)
    x_h = nc.declare_dram_parameter("x", [T, D], F32, isOutput=False)
    wr_h = nc.declare_dram_parameter("wr", [D, E], F32, isOutput=False)
    win_h = nc.declare_dram_parameter("w_in", [E, D, F], w_store, isOutput=False)
    wout_h = nc.declare_dram_parameter("w_out", [E, F, D], w_store, isOutput=False)
    br_h = nc.declare_dram_parameter("br", [1, E], F32, isOutput=False) if has_br else None
    bin_h = nc.declare_dram_parameter("b_in", [E, F], F32, isOutput=False) if has_bin else None
    bout_h = nc.declare_dram_parameter("b_out", [E, D], F32, isOutput=False) if has_bout else None
    y_h = nc.declare_dram_parameter("y", [T, D], F32, isOutput=True)

    with tile.TileContext(nc) as tc:
        with (
            tc.tile_pool(name="persist", bufs=1) as pp,
            tc.tile_pool(name="ps", bufs=6, space="PSUM") as psp,
        ):
            ident = pp.tile([P, P], F32, tag="ident")
            from concourse.masks import make_identity
            make_identity(nc, ident[:])

            xT = pp.tile([P, KD, T], F32, tag="xT")          # x transposed, f32
            hT = pp.tile([P, FT, T], w_store, tag="hT")      # h transposed
            xTr = pp.tile([P, KD, T], w_store, tag="xTr", name="xTr")
            wr_sb = pp.tile([P, KD, E], F32, tag="wr")
            disp = pp.tile([P, G * E], F32, tag="disp")      # dispatch mask
            comb = pp.tile([P, G * E], F32, tag="comb")      # combine probs
            yac = [
                pp.tile([P, D], F32, tag=f"y{g}", name=f"yac{g}")
                for g in range(G)
            ]
            ones1 = pp.tile([1, P], F32, tag="ones1")
            if has_bin or has_bout:
                nc.vector.memset(ones1[:], 1.0)
            br_sb = None
            if has_br:
                br_sb = pp.tile([1, E], F32, tag="br")
                nc.sync.dma_start(br_sb[:], br_h[:])

            nc.sync.dma_start(
                wr_sb[:], wr_h[:, :].rearrange("(kd p) e -> p kd e", p=P)
            )

            with tc.tile_pool(name="xload", bufs=2) as xlp:
                for g in range(G):
                    xg = xlp.tile([P, D], F32, tag="xg")
                    nc.sync.dma_start(xg[:], x_h[g * P : (g + 1) * P, :])
                    for kd in range(KD):
                        pst = psp.tile([P, P], F32, tag="ps")
                        nc.tensor.transpose(
                            pst[:], xg[:, kd * P : (kd + 1) * P], ident[:]
                        )
                        nc.vector.tensor_copy(
                            xT[:, kd, g * P : (g + 1) * P], pst[:]
                        )
                        nc.vector.tensor_copy(
                            xTr[:, kd, g * P : (g + 1) * P], pst[:]
                        )

            # router (true fp32 matmul; top-2 must match reference)
            with tc.tile_pool(name="rt", bufs=2) as rtp:
                for g in range(G):
                    psr = psp.tile([P, E], F32, tag="ps")
                    for kd in range(KD):
                        nc.tensor.matmul(
                            psr[:],
                            lhsT=xT[:, kd, g * P : (g + 1) * P],
                            rhs=wr_sb[:, kd, :],
                            start=(kd == 0),
                            stop=(kd == KD - 1 and not has_br),
                        )
                    if has_br:
                        nc.tensor.matmul(
                            psr[:], lhsT=ones1[:, :], rhs=br_sb[:, :],
                            start=False, stop=True,
                        )
                    lg = rtp.tile([P, E], F32, tag="lg")
                    nc.vector.tensor_copy(lg[:], psr[:])
                    mx1 = rtp.tile([P, 1], F32, tag="mx1")
                    nmx = rtp.tile([P, 1], F32, tag="nmx")
                    nc.vector.reduce_max(out=mx1[:], in_=lg[:], axis=AX.X)
                    nc.vector.reduce_max(out=nmx[:], in_=lg[:], axis=AX.X, negate=True)
                    is1 = rtp.tile([P, E], F32, tag="is1")
                    nc.vector.tensor_scalar(
                        out=is1[:], in0=lg[:], scalar1=mx1[:, :1], scalar2=None,
                        op0=OP.is_equal,
                    )
                    lgm = rtp.tile([P, E], F32, tag="lgm")
                    nc.vector.tensor_scalar_mul(is1[:], is1[:], 1e30)
                    nc.vector.tensor_sub(lgm[:], lg[:], is1[:])
                    mx2 = rtp.tile([P, 1], F32, tag="mx2")
                    nc.vector.reduce_max(out=mx2[:], in_=lgm[:], axis=AX.X)
                    dcol = disp[:, g * E : (g + 1) * E]
                    nc.vector.tensor_scalar(
                        out=dcol, in0=lg[:], scalar1=mx2[:, :1], scalar2=None,
                        op0=OP.is_ge,
                    )
                    ex = rtp.tile([P, E], F32, tag="ex")
                    nc.scalar.activation(ex[:], lg[:], AF.Exp, bias=nmx[:, :1])
                    sm = rtp.tile([P, 1], F32, tag="sm")
                    nc.vector.reduce_sum(out=sm[:], in_=ex[:], axis=AX.X)
                    rc = rtp.tile([P, 1], F32, tag="rc")
                    nc.vector.reciprocal(rc[:], sm[:])
                    nc.vector.tensor_scalar_mul(ex[:], ex[:], rc[:, :1])
                    nc.vector.tensor_mul(
                        comb[:, g * E : (g + 1) * E], ex[:], dcol
                    )

            # mm1: h = sum_e mask_e * relu(x@W_in[e] (+ b_in))
            with (
                tc.tile_pool(name="wfe", bufs=2) as wfp,
                tc.tile_pool(name="hf", bufs=2 * G) as hfp,
                tc.tile_pool(name="rtmp", bufs=4) as rtmp,
            ):
                for f in range(FC):
                    hfs = []
                    for e in range(E):
                        wfe = wfp.tile([P, KD, 512], w_store, tag="wfe")
                        nc.sync.dma_start(
                            wfe[:],
                            win_h[e, :, f * 512 : (f + 1) * 512].rearrange(
                                "(kd p) f -> p kd f", p=P
                            ),
                        )
                        if has_bin:
                            bin_sb = wfp.tile([1, 512], F32, tag="bin")
                            nc.sync.dma_start(
                                bin_sb[:],
                                bin_h[e, f * 512 : (f + 1) * 512][None, :],
                            )
                        for g in range(G):
                            ps = psp.tile([P, 512], F32, tag="ps")
                            for kd in range(KD):
                                nc.tensor.matmul(
                                    ps[:],
                                    lhsT=xTr[:, kd, g * P : (g + 1) * P],
                                    rhs=wfe[:, kd, :],
                                    start=(kd == 0),
                                    stop=(kd == KD - 1 and not has_bin),
                                )
                            if has_bin:
                                nc.tensor.matmul(
                                    ps[:],
                                    lhsT=ones1[:, :],
                                    rhs=bin_sb[:, :],
                                    start=False, stop=True,
                                )
                            sc = disp[:, g * E + e : g * E + e + 1]
                            if e == 0:
                                hf = hfp.tile([P, 512], F32, tag="hf")
                                hfs.append(hf)
                                nc.scalar.activation(
                                    hf[:], ps[:], AF.Relu, scale=sc
                                )
                            else:
                                tmp = rtmp.tile([P, 512], F32, tag="rtmp")
                                nc.scalar.activation(
                                    tmp[:], ps[:], AF.Relu, scale=sc
                                )
                                nc.vector.tensor_add(hfs[g][:], hfs[g][:], tmp[:])
                    for g in range(G):
                        for c in range(4):
                            pst = psp.tile([P, P], F32, tag="ps")
                            nc.tensor.transpose(
                                pst[:],
                                hfs[g][:, c * P : (c + 1) * P],
                                ident[:],
                            )
                            nc.vector.tensor_copy(
                                hT[:, f * 4 + c, g * P : (g + 1) * P], pst[:]
                            )

            # mm2: y = sum_e comb_e * (h@W_out[e] (+ b_out))
            ndh = 2 if wdt == F16 else 4
            dw = D // ndh
            with tc.tile_pool(name="wo", bufs=2) as wop:
                for e in range(E):
                    for dh in range(ndh):
                        wo = wop.tile([P, FT, dw], w_store, tag="wo")
                        nc.sync.dma_start(
                            wo[:],
                            wout_h[e, :, dh * dw : (dh + 1) * dw].rearrange(
                                "(ft p) d -> p ft d", p=P
                            ),
                        )
                        if has_bout:
                            bout_sb = wop.tile([1, dw], F32, tag="bout")
                            nc.sync.dma_start(
                                bout_sb[:],
                                bout_h[e, dh * dw : (dh + 1) * dw][None, :],
                            )
                        for g in range(G):
                            ps = psp.tile([P, dw], F32, tag="ps")
                            for ft in range(FT):
                                nc.tensor.matmul(
                                    ps[:],
                                    lhsT=hT[:, ft, g * P : (g + 1) * P],
                                    rhs=wo[:, ft, :],
                                    start=(ft == 0),
                                    stop=(ft == FT - 1 and not has_bout),
                                )
                            if has_bout:
                                nc.tensor.matmul(
                                    ps[:],
                                    lhsT=ones1[:, :],
                                    rhs=bout_sb[:, :],
                                    start=False, stop=True,
                                )
                            cc = comb[:, g * E + e : g * E + e + 1]
                            ysl = yac[g][:, dh * dw : (dh + 1) * dw]
                            if e == 0:
                                nc.vector.tensor_scalar(
                                    out=ysl, in0=ps[:], scalar1=cc,
                                    scalar2=None, op0=OP.mult,
                                )
                            else:
                                tm = wop.tile([P, dw], F32, tag="ytmp")
                                nc.vector.tensor_scalar(
                                    out=tm[:], in0=ps[:], scalar1=cc,
                                    scalar2=None, op0=OP.mult,
                                )
                                nc.vector.tensor_add(ysl, ysl, tm[:])

            for g in range(G):
                nc.sync.dma_start(y_h[g * P : (g + 1) * P, :], yac[g][:])

    nc.compile()
    return nc


_NC_CACHE = {}


def get_nc(cfg_key):
    if cfg_key not in _NC_CACHE:
        cfg = dict(
            wdt=cfg_key[0], has_br=cfg_key[1], has_bin=cfg_key[2],
            has_bout=cfg_key[3],
        )
        _NC_CACHE[cfg_key] = build_nc(cfg)
    return _NC_CACHE[cfg_key]


def get_nc_v4(cap):
    key = ("v4", cap)
    if key not in _NC_CACHE:
        _NC_CACHE[key] = build_nc_v4(cap)
    return _NC_CACHE[key]


WDT_MODE = os.environ.get("MOE_WDT", "f16")


def make_in_maps(x, Wr, br, W_in, b_in, W_out, b_out, wdt_mode):
    xf = np.ascontiguousarray(np.asarray(x, np.float32).reshape(N_TOK, D))
    w_store_np = np.float32 if wdt_mode == "f32r" else np.float16
    win = np.ascontiguousarray(np.asarray(W_in, w_store_np))
    wout = np.ascontiguousarray(np.asarray(W_out, w_store_np))
    wr = np.ascontiguousarray(np.asarray(Wr, np.float32))
    has_br = bool(np.any(np.asarray(br) != 0))
    has_bin = bool(np.any(np.asarray(b_in) != 0))
    has_bout = bool(np.any(np.asarray(b_out) != 0))
    in_maps = []
    for c in range(NCORES):
        m = {
            "x": xf[c * T : (c + 1) * T],
            "wr": wr,
            "w_in": win,
            "w_out": wout,
        }
        if has_br:
            m["br"] = np.asarray(br, np.float32).reshape(1, E)
        if has_bin:
            m["b_in"] = np.asarray(b_in, np.float32)
        if has_bout:
            m["b_out"] = np.asarray(b_out, np.float32)
        in_maps.append(m)
    cfg_key = (wdt_mode, has_br, has_bin, has_bout)
    return cfg_key, in_maps


# v4 = expert-parallel host-dispatched (default); v1 = dense fallback
# (v1 also serves as the general path when b_in/b_out is nonzero)
IMPL = os.environ.get("MOE_IMPL", "v4")


def kernel(x, Wr, br, W_in, b_in, W_out, b_out, top_k):
    assert int(top_k) == 2, "kernel is specialized for top_k=2"
    if IMPL == "v4" and not (np.any(np.asarray(b_in)) or np.any(np.asarray(b_out))):
        xf = np.ascontiguousarray(np.asarray(x, np.float32).reshape(NT, D))
        idx_list, p_list, cap = route_v4(xf, Wr, br)
        in_maps = make_in_maps_v4(x, W_in, W_out, idx_list, p_list, cap)
        nc = get_nc_v4(cap)
        res = run_bass_kernel_spmd(nc, in_maps, list(range(NCORES)))
        y = np.zeros((NT, D), np.float32)
        for e in range(E):
            n = len(idx_list[e])
            ye = np.asarray(res.results[e]["yt"])  # [D, cap] f16
            y[idx_list[e]] += ye[:, :n].T.astype(np.float32)
        return y.reshape(4, 1024, 1024)
    cfg_key, in_maps = make_in_maps(
        x, Wr, br, W_in, b_in, W_out, b_out, WDT_MODE
    )
    nc = get_nc(cfg_key)
    res = run_bass_kernel_spmd(nc, in_maps, list(range(NCORES)))
    y = np.concatenate([res.results[c]["y"] for c in range(NCORES)], axis=0)
    return y.reshape(4, 1024, 1024).astype(np.float32)
